# revision 1
# baseline (speedup 1.0000x reference)
"""Trainium2 Bass kernel for nn_LogicConv3d (differentiable logic-gate 3D conv).

Strategy
--------
The reference's big gather `x.reshape(B,-1)[:, lin]` is, structurally, reading
shifted 30x30x30 windows of the (C,32,32,32) volume: coords lie in [0,3), so
each (j,k,s) leaf operand is one of 81 shifted slices (c,dh,dw,dd).  Each tree
node is a bilinear blend  out = c0 + ca*a + cb*b + cab*a*b  whose coefficients
come from softmax(w)@GATES — tiny, computed on host.

Sharding: kernels K=32 are split 4-per-core across 8 cores (batch stays packed
into the partition/flat-position dimension).  Per-core differences are pure
DATA (pre-gathered operand slices + per-node coefficient columns), so a single
SPMD program runs on all 8 cores via run_bass_kernel_spmd.

Device layout: all B*P = 4*27000 = 108000 output positions are flattened into
(128 partitions, 844 free) tiles.  Per node (3 ops, fp16):
    u = tensor_scalar(b, cab, ca)            # u = cab*b + ca
    t = scalar_tensor_tensor(a, _, u, mult)  # t = a*u
    o = scalar_tensor_tensor(b, cb, t, add)  # o = cb*b + t
Each node's additive constant is folded into its parent's coefficients on the
host (the bilinear form is closed under constant shifts of its inputs); the
root constant is added by the final fp16->fp32 conversion op.
"""
import numpy as np

# ---- problem constants (hardcoded per contest contract) ----
B, C, H, W, D = 4, 3, 32, 32, 32
K, S = 32, 16
OH = OW = OD = 30
P = OH * OW * OD            # 27000
BP = B * P                  # 108000
NPART = 128
FREE = (BP + NPART - 1) // NPART   # 844
PADBP = NPART * FREE        # 108032
NCORES = 8
KLOC = K // NCORES          # 4
TEMP = 1.0
NLEV = 5
NODES_PER_K = 31            # 16+8+4+2+1
NNODES = KLOC * NODES_PER_K  # 124 per core
NCOLS = NNODES * 3 + KLOC   # coef columns: 3 per node + root consts

GATES = np.array([[(g >> t) & 1 for t in range(4)] for g in range(16)],
                 dtype=np.float64)

# engine assignment knobs (tuned after profiling)
TS_ACT_MOD = 12     # TS ops: ACT unless (node_idx % TS_ACT_MOD == 0) -> DVE
STT_GPS_MOD = 4     # STT ops: t-op to GPSIMD when idx%4==0, o-op when idx%4==2
USE_ACT = True
USE_GPS = False


# ----------------------------------------------------------------- host math
def _lut_coeffs(w):
    """w: (nodes,K,16) -> c0, ca, cb, cab each (nodes,K) float64."""
    w = w.astype(np.float64)
    e = np.exp((w - w.max(-1, keepdims=True)) / TEMP)
    p = e / e.sum(-1, keepdims=True)
    l = p @ GATES
    l0, l1, l2, l3 = l[..., 0], l[..., 1], l[..., 2], l[..., 3]
    return l0, l2 - l0, l1 - l0, l0 - l1 - l2 + l3


def _fold_coeffs(ws):
    """Fold per-node constants into parents.  Returns (folded, root_const):
    folded[lev] = (ca2, cb2, cab) each (nodes,K); root_const (K,)."""
    folded = []
    gamma = None
    for lev, w in enumerate(ws):
        c0, ca, cb, cab = _lut_coeffs(w)
        if lev == 0:
            gA = np.zeros_like(c0)
            gB = np.zeros_like(c0)
        else:
            gA = gamma[0::2]
            gB = gamma[1::2]
        folded.append((ca + cab * gB, cb + cab * gA, cab))
        gamma = c0 + ca * gA + cb * gB + cab * gA * gB
    return folded, gamma[0]


def _prep_inputs(x, kc, ws):
    """Build per-core in_maps (numpy)."""
    # 81 shifted windows, flattened positions (b,oh,ow,od), fp16, padded
    X81 = np.empty((3, 3, 3, 3, B, OH, OW, OD), np.float32)
    for c in range(3):
        for dh in range(3):
            for dw in range(3):
                for dd in range(3):
                    X81[c, dh, dw, dd] = x[:, c, dh:dh + 30, dw:dw + 30, dd:dd + 30]
    X81f = np.zeros((81, PADBP), np.float16)
    X81f[:, :BP] = X81.reshape(81, BP).astype(np.float16)
    X81f = X81f.reshape(81, NPART, FREE)

    h_, w_, d_, c_ = kc[..., 0], kc[..., 1], kc[..., 2], kc[..., 3]
    sl = ((c_ * 3 + h_) * 3 + w_) * 3 + d_          # (2,K,S)

    folded, root_const = _fold_coeffs(ws)

    in_maps = []
    for core in range(NCORES):
        ks = range(core * KLOC, (core + 1) * KLOC)
        a_in = np.ascontiguousarray(
            X81f[sl[0, ks].reshape(-1)])             # (64,128,FREE)
        b_in = np.ascontiguousarray(
            X81f[sl[1, ks].reshape(-1)])
        coef = np.zeros((NPART, NCOLS), np.float32)
        col = 0
        for kk, k in enumerate(ks):
            for lev in range(NLEV):
                ca2, cb2, cab = folded[lev]
                for i in range(ca2.shape[0]):
                    coef[:, col + 0] = cab[i, k]
                    coef[:, col + 1] = ca2[i, k]
                    coef[:, col + 2] = cb2[i, k]
                    col += 3
        for kk, k in enumerate(ks):
            coef[:, NNODES * 3 + kk] = root_const[k]
        in_maps.append({"a_in": a_in, "b_in": b_in, "coef": coef})
    return in_maps


# ------------------------------------------------------------ device program
def _build_program():
    import concourse.bass as bass
    import concourse.bacc as bacc
    import concourse.mybir as mybir
    from concourse.tile import TileContext

    f16 = mybir.dt.float16
    f32 = mybir.dt.float32
    Alu = mybir.AluOpType
    Act = mybir.ActivationFunctionType

    nc = bacc.Bacc()
    a_in = nc.declare_dram_parameter("a_in", [KLOC * S, NPART, FREE], f16,
                                     isOutput=False)
    b_in = nc.declare_dram_parameter("b_in", [KLOC * S, NPART, FREE], f16,
                                     isOutput=False)
    coef = nc.declare_dram_parameter("coef", [NPART, NCOLS], f32,
                                     isOutput=False)
    out = nc.declare_dram_parameter("out", [KLOC, NPART, FREE], f32,
                                    isOutput=True)

    node_idx = 0

    with TileContext(nc) as tc:
        with (
            tc.tile_pool(name="cpool", bufs=1) as cpool,
            tc.tile_pool(name="iopool", bufs=6) as iopool,
            tc.tile_pool(name="wpool", bufs=6) as wpool,
            tc.tile_pool(name="lpool", bufs=2) as lpool,
            tc.tile_pool(name="opool", bufs=3) as opool,
        ):
            coef_sb = cpool.tile([NPART, NCOLS], f32)
            nc.sync.dma_start(out=coef_sb[:], in_=coef[:])

            def node_eval(a_t, b_t, col, lev):
                nonlocal node_idx
                cab_ap = coef_sb[:, col:col + 1]
                ca_ap = coef_sb[:, col + 1:col + 2]
                cb_ap = coef_sb[:, col + 2:col + 3]
                u = wpool.tile([NPART, FREE], f16, tag="u", name=f"u{node_idx}")
                if USE_ACT and (node_idx % TS_ACT_MOD) != 0:
                    nc.scalar.activation(u[:], b_t[:], Act.Identity,
                                         bias=ca_ap, scale=cab_ap)
                else:
                    nc.vector.tensor_scalar(u[:], b_t[:], cab_ap, ca_ap,
                                            Alu.mult, Alu.add)
                t = wpool.tile([NPART, FREE], f16, tag="t", name=f"t{node_idx}")
                if USE_GPS and (node_idx % STT_GPS_MOD) == 0:
                    nc.gpsimd.scalar_tensor_tensor(
                        t[:], a_t[:], 0.0, u[:], Alu.bypass, Alu.mult)
                else:
                    nc.vector.scalar_tensor_tensor(
                        t[:], a_t[:], 0.0, u[:], Alu.bypass, Alu.mult)
                o = lpool.tile([NPART, FREE], f16, tag=f"o{lev}",
                               name=f"o{node_idx}", bufs=(18 >> lev) + 2)
                if USE_GPS and (node_idx % STT_GPS_MOD) == 2:
                    nc.gpsimd.scalar_tensor_tensor(
                        o[:], b_t[:], cb_ap, t[:], Alu.mult, Alu.add)
                else:
                    nc.vector.scalar_tensor_tensor(
                        o[:], b_t[:], cb_ap, t[:], Alu.mult, Alu.add)
                node_idx += 1
                return o

            for kk in range(KLOC):
                col0 = kk * NODES_PER_K * 3
                cur = []
                for s in range(S):
                    a_t = iopool.tile([NPART, FREE], f16, tag="ain",
                                      name=f"a{kk}_{s}")
                    nc.sync.dma_start(out=a_t[:], in_=a_in[kk * S + s])
                    b_t = iopool.tile([NPART, FREE], f16, tag="bin",
                                      name=f"b{kk}_{s}")
                    nc.sync.dma_start(out=b_t[:], in_=b_in[kk * S + s])
                    cur.append(node_eval(a_t, b_t, col0 + s * 3, 0))
                coff = 16
                for lev in range(1, NLEV):
                    nxt = []
                    for i in range(len(cur) // 2):
                        nxt.append(node_eval(
                            cur[2 * i], cur[2 * i + 1],
                            col0 + (coff + i) * 3, lev))
                    coff += len(nxt)
                    cur = nxt
                root_ap = coef_sb[:, NNODES * 3 + kk:NNODES * 3 + kk + 1]
                ot = opool.tile([NPART, FREE], f32, tag="out", name=f"ot{kk}")
                nc.vector.tensor_scalar(ot[:], cur[0][:], root_ap, None,
                                        Alu.add)
                nc.sync.dma_start(out=out[kk], in_=ot[:])
    nc.compile()
    return nc


_PROGRAM = None


def kernel(**inputs):
    global _PROGRAM
    x = np.asarray(inputs["x"], dtype=np.float32)
    kc = np.asarray(inputs["kernel_coords"])
    ws = [np.asarray(inputs[f"w{i}"]) for i in range(5)]

    in_maps = _prep_inputs(x, kc, ws)

    from concourse.bass_utils import run_bass_kernel_spmd
    if _PROGRAM is None:
        _PROGRAM = _build_program()
    res = run_bass_kernel_spmd(_PROGRAM, in_maps, list(range(NCORES)))
    results = res.results

    full = np.empty((K, PADBP), np.float32)
    for core in range(NCORES):
        o = results[core]["out"].reshape(KLOC, PADBP)
        full[core * KLOC:(core + 1) * KLOC] = o
    out = full[:, :BP].reshape(K, B, OH, OW, OD).transpose(1, 0, 2, 3, 4)
    return np.ascontiguousarray(out)

